# revision 25
# baseline (speedup 1.0000x reference)
"""Trainium2 Bass kernel for nn_ConditionPooler (ragged cross-attention pooler).

Algorithm (per core, data-parallel over B=16 scenes, S=2 scenes/core on 8 cores):
  scores^T[n,(h,t)] = feat @ A         where A[c,(h,t)] = sum_d qh[t,h,d] w_k[h*DH+d,c]
  P = exp(scores)  (no max-subtraction; scores in [-9, 9], fp16 exp is exact
                    enough; b_k_in cancels in softmax)
  U[(h,t),c]  = sum_n P[n,(h,t)] feat[n,c]   (per scene; padded rows are 0)
  den[(h,t)]  = sum_n P[n,(h,t)] mask[n]
  Uhat = U / den; attn_h = Uhat_h @ w_v_h^T; out = attn @ w_o^T + b_o (+ w_o b_v_in)
  h = out + query; z = (h-mu)/std; ff = gelu(z @ (w1*g)^T + b1eff) @ w2^T + b2
  result = h + ff

Performance design (v2): the previous build was PE-sequencer bound (~124ns of
PE.SEQ per matmul call, 515 calls).  This build:
  - feat is DMA'd in BOTH orientations (row-major fp16 for the U matmul's
    moving operand, transposed fp16 for the scores matmul's stationary
    operand) so there are NO on-chip PE transposes in the stream.
  - fp16 streaming operands (quantization error ~6e-4, final rel err ~baseline).
  - 7 matmul calls per 128-row tile: 4 scores (c-block chains), 2 U (ht
    halves), 1 denominator (mask-stationary, accumulating [1, 256] per scene).
  - U/den calls for pair p are emitted after the scores of pair p+1 so the
    PE never stalls on the exp (Activation engine) latency.
Scenes are padded to a common length P (multiple of 256*gch) so the SPMD
program is static; segment boundaries come from batch_idx on the host.
"""

import numpy as np

DEN_POOL = False

C = 512
T = 32
H = 8
DH = C // H
NCORES = 8

_CACHE = {}


def _apply_tile_patch():
    """This walrus build allows only one sem wait on CTRL-encoded (Drain)
    instructions; TileContext's tail drain carries the whole global clock.
    Split the extra waits onto standalone sync-engine nops."""
    import concourse.tile as tile_mod
    import concourse.mybir as mybir
    from concourse.vector_clock import ScopedClock

    if getattr(tile_mod.TileContext, "_drain_patched", False):
        return

    def _patched(self, tick_clock, wait_clock):
        nc = self.nc
        drain_inst = nc.sync.drain()
        wait_clock.add_sem_waits(
            drain_inst.ins, ScopedClock({None: tick_clock.global_clock})
        )
        si = drain_inst.ins.sync_info
        if si is not None and si.on_wait is not None and len(si.on_wait) > 1:
            waits = list(si.on_wait)
            si.on_wait = waits[:1]
            for w in waits[1:]:
                nop = nc.sync.nop(nofuse=True)
                nsi = nop.ins.sync_info
                if nsi is None:
                    nop.ins.sync_info = mybir.SyncInfo(on_wait=[w], on_update=[])
                else:
                    nsi.on_wait = [w]
        nc.all_engine_barrier()
        assert self.sems is not None
        popped = nc._tile_sem_poison_stack.pop()
        assert popped is self._sem_poison
        nc.clear_and_free_semaphores(list(self.sems.allocated().values()))
        nc.all_engine_barrier()

    tile_mod.TileContext._drain_and_barrier = _patched
    tile_mod.TileContext._drain_patched = True


def _split_multi_waits(nc):
    """This walrus build caps sync waits at 1 per instruction (2 for
    EventSemaphore). Tile emits several on some instructions; hoist the
    extras onto same-engine NoOps inserted just before."""
    import concourse.mybir as mybir

    cnt = [0]
    for f in nc.m.functions:
        for b in f.blocks:
            newlist = []
            for inst in b.instructions:
                si = inst.sync_info
                if si is not None and si.on_wait is not None and len(si.on_wait) > 1:
                    waits = list(si.on_wait)
                    for w in waits[:-1]:
                        cnt[0] += 1
                        nop = mybir.InstNoOp(
                            name=f"I-wsplit-{cnt[0]}", ins=[], outs=[]
                        )
                        nop.engine = inst.engine
                        nop.sync_info = mybir.SyncInfo(on_wait=[w], on_update=[])
                        newlist.append(nop)
                    si.on_wait = waits[-1:]
                newlist.append(inst)
            b.instructions = newlist


def _build(P, S, gch=3, split=True):
    """Build the per-core SPMD Bass program. P = padded scene length
    (multiple of 256*gch), S = scenes per core, gch = tile-pairs per DMA
    chunk."""
    import concourse.bass as bass
    import concourse.bass_isa as bass_isa
    import concourse.mybir as mybir
    import concourse.tile as tile

    _apply_tile_patch()

    f32 = mybir.dt.float32
    f16 = mybir.dt.float16
    HT = H * T  # 256
    NT = P // 128
    NP = NT // 2  # tile pairs per scene
    NCH = NP // gch  # DMA chunks per scene
    assert P % (256 * gch) == 0
    AF = mybir.ActivationFunctionType
    ALU = mybir.AluOpType

    nc = bass.Bass()
    featp = nc.dram_tensor("featp", [S * P, C], f16, kind="ExternalInput")
    featTp = nc.dram_tensor("featTp", [C, S * P], f16, kind="ExternalInput")
    akT_d = nc.dram_tensor("akT", [C, HT], f16, kind="ExternalInput")
    wvT_d = nc.dram_tensor("wvT", [C, C], f16, kind="ExternalInput")
    woT_d = nc.dram_tensor("woT", [C, C], f16, kind="ExternalInput")
    w1gT_d = nc.dram_tensor("w1gT", [C, 2 * C], f16, kind="ExternalInput")
    b1e_d = nc.dram_tensor("b1e", [1, 2 * C], f16, kind="ExternalInput")
    w2T_d = nc.dram_tensor("w2T", [2 * C, C], f16, kind="ExternalInput")
    b2e_d = nc.dram_tensor("b2e", [1, C], f16, kind="ExternalInput")
    qb_d = nc.dram_tensor("qb", [T, C], f32, kind="ExternalInput")
    idh_d = nc.dram_tensor("identh", [128, 128], f16, kind="ExternalInput")
    id32_d = nc.dram_tensor("ident32", [128, 128], f32, kind="ExternalInput")
    outp = nc.dram_tensor("outp", [S * T, C], f32, kind="ExternalOutput")

    ST = S * T

    with tile.TileContext(nc) as tc:
        with tc.tile_pool(name="const", bufs=1) as const:
            # identh first (the warmup matmuls need it), then akT (scores)
            identh = const.tile([128, 128], f16, tag="identh")
            nc.sync.dma_start(identh[:], idh_d[:])
            akT = const.tile([128, 4, HT], f16, tag="akT")
            nc.sync.dma_start(akT[:], akT_d.rearrange("(j p) f -> p j f", p=128))
            wvT = const.tile([128, 4, C], f16, tag="wvT")
            woT = const.tile([128, 4, C], f16, tag="woT")
            w1gT = const.tile([128, 4, 2 * C], f16, tag="w1gT")
            w2T = const.tile([128, 8, C], f16, tag="w2T")
            b1e = const.tile([1, 2 * C], f16, tag="b1e")
            b2e = const.tile([1, C], f16, tag="b2e")
            qb2 = const.tile([ST, C], f32, tag="qb2")
            id32 = const.tile([128, 128], f32, tag="id32")

            # Epilogue weights are loaded in pieces, one per feat-chunk
            # boundary on the same (in-order) queue as the feat stream, so
            # they never displace more than one chunk-slack of feat DMA.
            wv_v = wvT_d.rearrange("(j p) f -> p j f", p=128)
            wo_v = woT_d.rearrange("(j p) f -> p j f", p=128)
            w1_v = w1gT_d.rearrange("(j p) f -> p j f", p=128)
            w2_v = w2T_d.rearrange("(j p) f -> p j f", p=128)
            epi_pieces = [
                lambda: nc.sync.dma_start(wvT[:], wv_v),
                lambda: nc.sync.dma_start(woT[:], wo_v),
                lambda: nc.sync.dma_start(w1gT[:, :, 0:C], w1_v[:, :, 0:C]),
                lambda: nc.sync.dma_start(w1gT[:, :, C:], w1_v[:, :, C:]),
                lambda: nc.sync.dma_start(w2T[:, 0:4, :], w2_v[:, 0:4, :]),
                lambda: nc.sync.dma_start(w2T[:, 4:8, :], w2_v[:, 4:8, :]),
            ]

            def load_epi_small():
                g = nc.gpsimd
                g.dma_start(b1e[:], b1e_d[:])
                g.dma_start(b2e[:], b2e_d[:])
                g.dma_start(id32[:], id32_d[:])
                for s in range(S):
                    g.dma_start(qb2[s * T : (s + 1) * T, :], qb_d[:])

            ones_col = const.tile([128, 1], f16, tag="ones_col")
            nc.vector.memset(ones_col[:], 1.0)
            ones = const.tile([1, ST], f16, tag="ones")
            nc.vector.memset(ones[:], 1.0)

            featv = featp.rearrange(
                "(s g q t p) c -> s g p q t c", p=128, t=2, q=gch, g=NCH
            )
            featTv = featTp.rearrange(
                "(j p) (s g n) -> s g p j n", p=128, g=NCH, n=gch * 256
            )

            with tc.tile_pool(name="epiA", bufs=1) as epiA:
                rden = epiA.tile([128, 2, S], f32, tag="rden")
                den_sb = epiA.tile([1, S, HT], f32, tag="den_sb")
                dacc = [
                    epiA.tile([1, 2, HT], f32, tag=f"dacc{s}", name=f"dacc{s}")
                    for s in range(S)
                ]
                Uhat = [
                    epiA.tile([128, 2, C], f16, tag=f"Uh{s}", name=f"Uh{s}")
                    for s in range(S)
                ]
                UT = epiA.tile([128, 4, S, HT], f16, tag="UT")
                with tc.tile_pool(name="pse", bufs=1, space="PSUM") as pse:
                    with (
                        tc.tile_pool(name="psU", bufs=1, space="PSUM") as psU_pool,
                        tc.tile_pool(name="psD", bufs=1, space="PSUM") as psD_pool,
                        tc.tile_pool(name="psDT", bufs=1, space="PSUM") as psDT_pool,
                        tc.tile_pool(name="fb", bufs=3) as fpool,
                        tc.tile_pool(name="ftb", bufs=3) as ftpool,
                        tc.tile_pool(name="pb", bufs=4) as ppool,
                        tc.tile_pool(name="rb", bufs=2) as rpool,
                        tc.tile_pool(name="psS", bufs=3, space="PSUM") as psS_pool,
                    ):
                        def emit_U(st):
                            PT2p, Fcp, qp, sp, pairp, Upsp, dpsp = st
                            for t in range(2):
                                ti = 2 * pairp + t
                                for h2 in range(2):
                                    nc.tensor.matmul(
                                        Upsp[h2][:],
                                        PT2p[:, t, h2 * 128 : (h2 + 1) * 128],
                                        Fcp[:, qp, t, :],
                                        start=(ti == 0),
                                        stop=(ti == NT - 1),
                                    )
                            if not DEN_POOL:
                                nc.tensor.matmul(
                                    dpsp[:],
                                    ones_col[:],
                                    PT2p[:],
                                    start=(pairp == 0),
                                    stop=(pairp == NP - 1),
                                )

                        def finish_scene(st):
                            sp, Upsp, dpsp = st[3], st[5], st[6]
                            if not DEN_POOL:
                                # DVE may read only one PSUM operand: stage
                                # the den accumulator in SBUF before folding
                                nc.vector.tensor_copy(dacc[sp][:], dpsp[:])
                            nc.vector.tensor_add(
                                den_sb[:, sp, :],
                                dacc[sp][:, 0, :],
                                dacc[sp][:, 1, :],
                            )
                            dT = psDT_pool.tile(
                                [128, 2], f32, tag="dT", name=f"dT{sp}"
                            )
                            for h2 in range(2):
                                nc.tensor.transpose(
                                    dT[:, h2 : h2 + 1],
                                    den_sb[:, sp, h2 * 128 : (h2 + 1) * 128],
                                    id32[:1, :1],
                                )
                            nc.vector.reciprocal(rden[:, :, sp], dT[:])
                            for h2 in range(2):
                                nc.vector.tensor_scalar_mul(
                                    Uhat[sp][:, h2, :],
                                    Upsp[h2][:],
                                    rden[:, h2, sp : sp + 1],
                                )
                            # transpose Uhat -> UT while the next scene streams
                            ps_u = pse.tile(
                                [128, 4, HT], f16, tag="tre", name=f"ps_u{sp}"
                            )
                            for h2 in range(2):
                                for jc in range(4):
                                    nc.tensor.transpose(
                                        ps_u[:, jc, h2 * 128 : (h2 + 1) * 128],
                                        Uhat[sp][:, h2, jc * 128 : (jc + 1) * 128],
                                        identh[:],
                                    )
                            nc.any.tensor_copy(UT[:, :, sp, :], ps_u[:])

                        def flush(st):
                            emit_U(st)
                            if st[4] == NP - 1:  # last pair of its scene
                                finish_scene(st)

                        # PE warmup: ~40 independent matmuls on the identity
                        # keep the PE busy through the first chunk DMAs and
                        # bring it to the full-speed p-state before the
                        # real stream begins.
                        warm = psU_pool.tile([128, C], f32, tag="U0", name="warm")
                        for _ in range(40):
                            nc.tensor.matmul(
                                warm[:, 0:128],
                                identh[:],
                                identh[:],
                                start=True,
                                stop=True,
                            )

                        pend = []
                        for s in range(S):
                            Ups = [
                                psU_pool.tile(
                                    [128, C], f32, tag=f"U{h2}", name=f"U{s}{h2}"
                                )
                                for h2 in range(2)
                            ]
                            dps = psD_pool.tile(
                                [1, 2, HT], f32, tag="den", name=f"d{s}"
                            )
                            for g in range(NCH):
                                FTc = ftpool.tile(
                                    [128, 4, gch * 256], f16, tag="FT"
                                )
                                nc.sync.dma_start(FTc[:], featTv[s, g])
                                Fc = fpool.tile([128, gch, 2, C], f16, tag="F")
                                nc.sync.dma_start(Fc[:], featv[s, g])
                                ci = s * NCH + g
                                if ci == 0:
                                    load_epi_small()
                                elif epi_pieces:
                                    epi_pieces.pop(0)()
                                for q in range(gch):
                                    pair = g * gch + q
                                    ps_s = psS_pool.tile(
                                        [128, 2, HT], f32, tag="sc"
                                    )
                                    for t in range(2):
                                        n0 = (2 * q + t) * 128
                                        for j in range(4):
                                            nc.tensor.matmul(
                                                ps_s[:, t, :],
                                                FTc[:, j, n0 : n0 + 128],
                                                akT[:, j, :],
                                                start=(j == 0),
                                                stop=(j == 3),
                                            )
                                    PT2 = ppool.tile([128, 2, HT], f16, tag="PT")
                                    nc.scalar.activation(PT2[:], ps_s[:], AF.Exp)
                                    if DEN_POOL:
                                        red = rpool.tile(
                                            [128, 2, HT], f32, tag="red"
                                        )
                                        nc.gpsimd.partition_all_reduce(
                                            red[:],
                                            PT2[:],
                                            channels=128,
                                            reduce_op=bass_isa.ReduceOp.add,
                                        )
                                        if pair == 0:
                                            nc.vector.tensor_copy(
                                                dacc[s][:], red[0:1, :, :]
                                            )
                                        else:
                                            nc.vector.tensor_add(
                                                dacc[s][:],
                                                dacc[s][:],
                                                red[0:1, :, :],
                                            )
                                    if len(pend) == 2:
                                        flush(pend.pop(0))
                                    pend.append((PT2, Fc, q, s, pair, Ups, dps))
                        # any weight pieces not yet issued (few chunk
                        # boundaries): issue them now, before the epilogue
                        while epi_pieces:
                            epi_pieces.pop(0)()
                        for st in pend:
                            flush(st)
                    # ---- epilogue (streaming PSUM banks now free) ----
                    with (
                        tc.tile_pool(name="epiB", bufs=1) as epi,
                        tc.tile_pool(name="pacc", bufs=1, space="PSUM") as pacc,
                    ):
                        # attention value projection: attnT[(hd), (s,t)]
                        at_ps = pacc.tile([128, 4, S, T], f32, tag="at")
                        for gq in range(4):
                            for hh in range(2):
                                h = 2 * gq + hh
                                for jc in range(4):
                                    nc.tensor.matmul(
                                        at_ps[hh * 64 : (hh + 1) * 64, gq, :, :],
                                        wvT[:, jc, h * DH : (h + 1) * DH],
                                        UT[:, jc, :, h * T : (h + 1) * T],
                                        start=(jc == 0),
                                        stop=(jc == 3),
                                    )
                        at_sb = epi.tile([128, 4, S, T], f16, tag="at_sb")
                        nc.any.tensor_copy(at_sb[:], at_ps[:])

                        # output projection -> h = out + query + b
                        ph = pacc.tile([ST, C], f32, tag="ph")
                        for gq in range(4):
                            nc.tensor.matmul(
                                ph[:],
                                at_sb[:, gq, :, :],
                                woT[:, gq, :],
                                start=(gq == 0),
                                stop=(gq == 3),
                            )
                        h_sb = epi.tile([ST, C], f32, tag="h")
                        nc.vector.tensor_add(h_sb[:], ph[:], qb2[:])

                        # layernorm -> z (fp16): var = E[h^2] - mu^2, then
                        # z = h*rstd - mu*rstd in one fused two-scalar op.
                        ssum = epi.tile([ST, 1], f32, tag="ssum")
                        nc.vector.reduce_sum(
                            ssum[:], h_sb[:], axis=mybir.AxisListType.X
                        )
                        sq = epi.tile([ST, C], f32, tag="sq")
                        s2 = epi.tile([ST, 1], f32, tag="s2")
                        nc.scalar.activation(
                            sq[:], h_sb[:], AF.Square, accum_out=s2[:]
                        )
                        mu = epi.tile([ST, 1], f32, tag="mu")
                        nc.vector.tensor_scalar_mul(mu[:], ssum[:], 1.0 / C)
                        musq = epi.tile([ST, 1], f32, tag="musq")
                        nc.vector.tensor_tensor(musq[:], mu[:], mu[:], op=ALU.mult)
                        var = epi.tile([ST, 1], f32, tag="var")
                        nc.vector.tensor_scalar(
                            var[:],
                            s2[:],
                            1.0 / C,
                            musq[:],
                            op0=ALU.mult,
                            op1=ALU.subtract,
                        )
                        epsc = epi.tile([ST, 1], f32, tag="epsc")
                        nc.vector.memset(epsc[:], 1e-5)
                        std = epi.tile([ST, 1], f32, tag="std")
                        nc.scalar.activation(std[:], var[:], AF.Sqrt, bias=epsc[:])
                        rstd = epi.tile([ST, 1], f32, tag="rstd")
                        nc.vector.reciprocal(rstd[:], std[:])
                        murstd = epi.tile([ST, 1], f32, tag="murstd")
                        nc.vector.tensor_tensor(murstd[:], mu[:], rstd[:], op=ALU.mult)
                        z = epi.tile([ST, C], f16, tag="z")
                        nc.vector.tensor_scalar(
                            z[:],
                            h_sb[:],
                            rstd[:],
                            murstd[:],
                            op0=ALU.mult,
                            op1=ALU.subtract,
                        )

                        # zT
                        zT = epi.tile([128, 4, ST], f16, tag="zT")
                        ps_z = pse.tile([128, 4, ST], f16, tag="tre", name="ps_z")
                        for jc in range(4):
                            nc.tensor.transpose(
                                ps_z[:, jc, :],
                                z[:, jc * 128 : (jc + 1) * 128],
                                identh[:ST, :ST],
                            )
                        nc.any.tensor_copy(zT[:], ps_z[:])

                        # ff1 + gelu
                        gm = epi.tile([ST, 2, C], f16, tag="gm")
                        for half in range(2):
                            pf = pacc.tile([ST, C], f32, tag=f"pf{half}")
                            for jc in range(4):
                                nc.tensor.matmul(
                                    pf[:],
                                    zT[:, jc, :],
                                    w1gT[:, jc, half * C : (half + 1) * C],
                                    start=(jc == 0),
                                    stop=False,
                                )
                            nc.tensor.matmul(
                                pf[:],
                                ones[:],
                                b1e[:, half * C : (half + 1) * C],
                                start=False,
                                stop=True,
                            )
                            nc.scalar.activation(gm[:, half, :], pf[:], AF.Gelu)

                        # gmT
                        gmT = epi.tile([128, 8, ST], f16, tag="gmT")
                        for half in range(2):
                            ps_g = pse.tile(
                                [128, 4, ST], f16, tag="tre", name=f"ps_g{half}"
                            )
                            for jc in range(4):
                                nc.tensor.transpose(
                                    ps_g[:, jc, :],
                                    gm[:, half, jc * 128 : (jc + 1) * 128],
                                    identh[:ST, :ST],
                                )
                            nc.any.tensor_copy(
                                gmT[:, half * 4 : (half + 1) * 4, :], ps_g[:]
                            )

                        # ff2 + residual
                        po = pacc.tile([ST, C], f32, tag="po")
                        for k in range(8):
                            nc.tensor.matmul(
                                po[:],
                                gmT[:, k, :],
                                w2T[:, k, :],
                                start=(k == 0),
                                stop=False,
                            )
                        nc.tensor.matmul(po[:], ones[:], b2e[:], start=False, stop=True)
                        fin = epi.tile([ST, C], f32, tag="fin")
                        nc.vector.tensor_add(fin[:], h_sb[:], po[:])
                        nc.sync.dma_start(outp[:], fin[:])

    if split:
        _split_multi_waits(nc)
    return nc


def _pick_geometry(max_count):
    """Smallest padded scene length P (and its chunking gch) with
    P = k * 256 * gch >= max_count, preferring larger DMA chunks."""
    best = None
    for gch in (3, 2, 1):
        blk = 256 * gch
        Pg = ((max_count + blk - 1) // blk) * blk
        if best is None or Pg < best[0]:
            best = (Pg, gch)
    return best


def _host_prep(inputs):
    feat = np.asarray(inputs["feat"], dtype=np.float32)
    batch_idx = np.asarray(inputs["batch_idx"]).astype(np.int64)
    B = int(np.asarray(inputs["batch_size"]))
    query = np.asarray(inputs["query"], dtype=np.float32)
    g_q = np.asarray(inputs["g_q"], np.float32)
    b_q = np.asarray(inputs["b_q"], np.float32)
    w_q = np.asarray(inputs["w_q"], np.float32)
    w_k = np.asarray(inputs["w_k"], np.float32)
    w_v = np.asarray(inputs["w_v"], np.float32)
    b_q_in = np.asarray(inputs["b_q_in"], np.float32)
    b_v_in = np.asarray(inputs["b_v_in"], np.float32)
    w_o = np.asarray(inputs["w_o"], np.float32)
    b_o = np.asarray(inputs["b_o"], np.float32)
    g_ff = np.asarray(inputs["g_ff"], np.float32)
    b_ff = np.asarray(inputs["b_ff"], np.float32)
    w1 = np.asarray(inputs["w1"], np.float32)
    b1 = np.asarray(inputs["b1"], np.float32)
    w2 = np.asarray(inputs["w2"], np.float32)
    b2 = np.asarray(inputs["b2"], np.float32)

    S = B // NCORES
    counts = np.bincount(batch_idx, minlength=B)
    offs = np.concatenate([[0], np.cumsum(counts)])
    P, gch = _pick_geometry(int(counts.max()))
    NT = P // 128

    f16 = np.float16

    # query-side fold (host; tiny)
    q = query[0]
    mu = q.mean(-1, keepdims=True)
    var = ((q - mu) ** 2).mean(-1, keepdims=True)
    qn = (q - mu) / np.sqrt(var + 1e-5) * g_q + b_q
    qh = (qn @ w_q.T + b_q_in) / np.sqrt(DH)  # [T, C]
    A = np.einsum(
        "thd,hdc->cht", qh.reshape(T, H, DH), w_k.reshape(H, DH, C)
    ).reshape(C, H * T)

    # pad-column vector v with v @ A[:, ht] <= -22 for every ht: padded
    # featT columns score ~exp(-22)=0, so the denominator needs no mask.
    An = A / np.linalg.norm(A, axis=0, keepdims=True)
    u, *_ = np.linalg.lstsq(An.T, np.ones(H * T), rcond=None)
    if (An.T @ u).min() < 0.5:
        raise RuntimeError("pad-vector separation failed")

    featp = np.zeros((NCORES, S * P, C), dtype=f16)
    for b in range(B):
        c, s = divmod(b, S)
        n = counts[b]
        featp[c, s * P : s * P + n] = feat[offs[b] : offs[b + 1]].astype(f16)
    featTp = np.ascontiguousarray(featp.transpose(0, 2, 1))  # [NCORES, C, S*P]
    m = float((A.T @ u).min())
    v = (-(22.0 / m) * u).astype(f16)
    for b in range(B):
        c, s = divmod(b, S)
        n = counts[b]
        if n < P:
            featTp[c, :, s * P + n : (s + 1) * P] = v[:, None]

    consts = dict(
        akT=np.ascontiguousarray(A.astype(f16)),
        wvT=np.ascontiguousarray(w_v.T.astype(f16)),
        woT=np.ascontiguousarray(w_o.T.astype(f16)),
        w1gT=np.ascontiguousarray((w1 * g_ff[None, :]).T.astype(f16)),
        b1e=(b1 + w1 @ b_ff).reshape(1, 2 * C).astype(f16),
        w2T=np.ascontiguousarray(w2.T.astype(f16)),
        b2e=b2.reshape(1, C).astype(f16),
        qb=np.ascontiguousarray(query[0] + (b_o + w_o @ b_v_in)[None, :]).astype(
            np.float32
        ),
        identh=np.eye(128, dtype=f16),
        ident32=np.eye(128, dtype=np.float32),
    )
    in_maps = []
    for c in range(NCORES):
        m = dict(consts)
        m["featp"] = featp[c]
        m["featTp"] = featTp[c]
        in_maps.append(m)
    return in_maps, P, S, B, gch


def kernel(**inputs):
    from concourse.bass_utils import run_bass_kernel_spmd

    in_maps, P, S, B, gch = _host_prep(inputs)
    key = (P, S, gch)
    if key not in _CACHE:
        _CACHE[key] = _build(P, S, gch=gch)
    nc = _CACHE[key]
    res = run_bass_kernel_spmd(nc, in_maps, core_ids=list(range(NCORES)))
    out = np.empty((B, T, C), dtype=np.float32)
    for c in range(NCORES):
        o = res.results[c]["outp"]
        for s in range(S):
            out[c * S + s] = o[s * T : (s + 1) * T]
    return out


# revision 42
# speedup vs baseline: 1.0136x; 1.0136x over previous
"""Trainium2 Bass kernel for nn_ConditionPooler (ragged cross-attention pooler).

Algorithm (per core, data-parallel over B=16 scenes, S=2 scenes/core on 8 cores):
  scores^T[n,(h,t)] = feat @ A         where A[c,(h,t)] = sum_d qh[t,h,d] w_k[h*DH+d,c]
  P = exp(scores)  (no max-subtraction; scores in [-9, 9], fp16 exp is exact
                    enough; b_k_in cancels in softmax)
  U[(h,t),c]  = sum_n P[n,(h,t)] feat[n,c]   (per scene; padded rows are 0)
  den[(h,t)]  = sum_n P[n,(h,t)] mask[n]
  Uhat = U / den; attn_h = Uhat_h @ w_v_h^T; out = attn @ w_o^T + b_o (+ w_o b_v_in)
  h = out + query; z = (h-mu)/std; ff = gelu(z @ (w1*g)^T + b1eff) @ w2^T + b2
  result = h + ff

Performance design (v2): the previous build was PE-sequencer bound (~124ns of
PE.SEQ per matmul call, 515 calls).  This build:
  - feat is DMA'd in BOTH orientations (row-major fp16 for the U matmul's
    moving operand, transposed fp16 for the scores matmul's stationary
    operand) so there are NO on-chip PE transposes in the stream.
  - fp16 streaming operands (quantization error ~6e-4, final rel err ~baseline).
  - 7 matmul calls per 128-row tile: 4 scores (c-block chains), 2 U (ht
    halves), 1 denominator (mask-stationary, accumulating [1, 256] per scene).
  - U/den calls for pair p are emitted after the scores of pair p+1 so the
    PE never stalls on the exp (Activation engine) latency.
Scenes are padded to a common length P (multiple of 256*gch) so the SPMD
program is static; segment boundaries come from batch_idx on the host.
"""

import numpy as np

DEN_POOL = False

C = 512
T = 32
H = 8
DH = C // H
NCORES = 8

_CACHE = {}


def _apply_tile_patch():
    """This walrus build allows only one sem wait on CTRL-encoded (Drain)
    instructions; TileContext's tail drain carries the whole global clock.
    Split the extra waits onto standalone sync-engine nops."""
    import concourse.tile as tile_mod
    import concourse.mybir as mybir
    from concourse.vector_clock import ScopedClock

    if getattr(tile_mod.TileContext, "_drain_patched", False):
        return

    def _patched(self, tick_clock, wait_clock):
        nc = self.nc
        drain_inst = nc.sync.drain()
        wait_clock.add_sem_waits(
            drain_inst.ins, ScopedClock({None: tick_clock.global_clock})
        )
        si = drain_inst.ins.sync_info
        if si is not None and si.on_wait is not None and len(si.on_wait) > 1:
            waits = list(si.on_wait)
            si.on_wait = waits[:1]
            for w in waits[1:]:
                nop = nc.sync.nop(nofuse=True)
                nsi = nop.ins.sync_info
                if nsi is None:
                    nop.ins.sync_info = mybir.SyncInfo(on_wait=[w], on_update=[])
                else:
                    nsi.on_wait = [w]
        nc.all_engine_barrier()
        assert self.sems is not None
        popped = nc._tile_sem_poison_stack.pop()
        assert popped is self._sem_poison
        nc.clear_and_free_semaphores(list(self.sems.allocated().values()))
        nc.all_engine_barrier()

    tile_mod.TileContext._drain_and_barrier = _patched
    tile_mod.TileContext._drain_patched = True


def _split_multi_waits(nc):
    """This walrus build caps sync waits at 1 per instruction (2 for
    EventSemaphore). Tile emits several on some instructions; hoist the
    extras onto same-engine NoOps inserted just before."""
    import concourse.mybir as mybir

    cnt = [0]
    for f in nc.m.functions:
        for b in f.blocks:
            newlist = []
            for inst in b.instructions:
                si = inst.sync_info
                if si is not None and si.on_wait is not None and len(si.on_wait) > 1:
                    waits = list(si.on_wait)
                    for w in waits[:-1]:
                        cnt[0] += 1
                        nop = mybir.InstNoOp(
                            name=f"I-wsplit-{cnt[0]}", ins=[], outs=[]
                        )
                        nop.engine = inst.engine
                        nop.sync_info = mybir.SyncInfo(on_wait=[w], on_update=[])
                        newlist.append(nop)
                    si.on_wait = waits[-1:]
                newlist.append(inst)
            b.instructions = newlist


def _build(P, S, gch=3, split=True):
    """Build the per-core SPMD Bass program. P = padded scene length
    (multiple of 256*gch), S = scenes per core, gch = tile-pairs per DMA
    chunk."""
    import concourse.bass as bass
    import concourse.bass_isa as bass_isa
    import concourse.mybir as mybir
    import concourse.tile as tile

    _apply_tile_patch()

    f32 = mybir.dt.float32
    f16 = mybir.dt.float16
    HT = H * T  # 256
    NT = P // 128
    NP = NT // 2  # tile pairs per scene
    NCH = NP // gch  # DMA chunks per scene
    assert P % (256 * gch) == 0
    AF = mybir.ActivationFunctionType
    ALU = mybir.AluOpType

    nc = bass.Bass()
    f8 = mybir.dt.float8e3
    f8e4 = mybir.dt.float8e4
    featp = nc.dram_tensor("featp", [S * P, C], f16, kind="ExternalInput")
    featTp = nc.dram_tensor("featTp", [C, S * P], f16, kind="ExternalInput")
    akT_d = nc.dram_tensor("akT", [C, HT], f16, kind="ExternalInput")
    wvT_d = nc.dram_tensor("wvT", [C, C], f16, kind="ExternalInput")
    woT_d = nc.dram_tensor("woT", [C, C], f16, kind="ExternalInput")
    w1gT_d = nc.dram_tensor("w1gT", [C, 2 * C], f16, kind="ExternalInput")
    b1e_d = nc.dram_tensor("b1e", [1, 2 * C], f16, kind="ExternalInput")
    w2T_d = nc.dram_tensor("w2T", [2 * C, C], f16, kind="ExternalInput")
    b2e_d = nc.dram_tensor("b2e", [1, C], f16, kind="ExternalInput")
    qb_d = nc.dram_tensor("qb", [T, C], f32, kind="ExternalInput")
    idh_d = nc.dram_tensor("identh", [128, 128], f16, kind="ExternalInput")
    id32_d = nc.dram_tensor("ident32", [128, 128], f32, kind="ExternalInput")
    outp = nc.dram_tensor("outp", [S * T, C], f32, kind="ExternalOutput")

    ST = S * T

    with tile.TileContext(nc) as tc:
        with tc.tile_pool(name="const", bufs=1) as const:
            # akT first on the sync queue (scores need it); the warmup
            # matmuls run on a memset tile so they need no DMA at all
            akT = const.tile([128, 4, HT], f16, tag="akT")
            nc.sync.dma_start(akT[:], akT_d.rearrange("(j p) f -> p j f", p=128))
            identh = const.tile([128, 128], f16, tag="identh")
            wsrc = const.tile([128, 128], f16, tag="wsrc")
            nc.vector.memset(wsrc[:], 0.001)
            wvT = const.tile([128, 4, C], f16, tag="wvT")
            woT = const.tile([128, 4, C], f16, tag="woT")
            w1gT = const.tile([128, 4, 2 * C], f16, tag="w1gT")
            w2T = const.tile([128, 8, C], f16, tag="w2T")
            b1e = const.tile([1, 2 * C], f16, tag="b1e")
            b2e = const.tile([1, C], f16, tag="b2e")
            qb2 = const.tile([ST, C], f32, tag="qb2")
            id32 = const.tile([128, 128], f32, tag="id32")

            # Epilogue weights are loaded in pieces, one per feat-chunk
            # boundary on the same (in-order) queue as the feat stream, so
            # they never displace more than one chunk-slack of feat DMA.
            wv_v = wvT_d.rearrange("(j p) f -> p j f", p=128)
            wo_v = woT_d.rearrange("(j p) f -> p j f", p=128)
            w1_v = w1gT_d.rearrange("(j p) f -> p j f", p=128)
            w2_v = w2T_d.rearrange("(j p) f -> p j f", p=128)
            epi_pieces = [
                lambda: nc.sync.dma_start(wvT[:], wv_v),
                lambda: nc.sync.dma_start(woT[:], wo_v),
                lambda: nc.sync.dma_start(w1gT[:, :, 0:C], w1_v[:, :, 0:C]),
                lambda: nc.sync.dma_start(w1gT[:, :, C:], w1_v[:, :, C:]),
                lambda: nc.sync.dma_start(w2T[:, 0:4, :], w2_v[:, 0:4, :]),
                lambda: nc.sync.dma_start(w2T[:, 4:8, :], w2_v[:, 4:8, :]),
            ]

            def load_epi_small():
                g = nc.gpsimd
                g.dma_start(b1e[:], b1e_d[:])
                g.dma_start(b2e[:], b2e_d[:])
                g.dma_start(id32[:], id32_d[:])
                g.dma_start(identh[:], idh_d[:])
                for s in range(S):
                    g.dma_start(qb2[s * T : (s + 1) * T, :], qb_d[:])

            ones_col = const.tile([128, 1], f16, tag="ones_col")
            nc.vector.memset(ones_col[:], 1.0)
            ones = const.tile([1, ST], f16, tag="ones")
            nc.vector.memset(ones[:], 1.0)

            featv = featp.rearrange(
                "(s g q t p) c -> s g p q t c", p=128, t=2, q=gch, g=NCH
            )
            featTv = featTp.rearrange(
                "(j p) (s g n) -> s g p j n", p=128, g=NCH, n=gch * 256
            )

            with tc.tile_pool(name="epiA", bufs=1) as epiA:
                rden = epiA.tile([128, 2, S], f32, tag="rden")
                den_sb = epiA.tile([1, S, HT], f32, tag="den_sb")
                dacc = [
                    epiA.tile([1, 2, HT], f32, tag=f"dacc{s}", name=f"dacc{s}")
                    for s in range(S)
                ]
                Uhat = [
                    epiA.tile([128, 2, C], f16, tag=f"Uh{s}", name=f"Uh{s}")
                    for s in range(S)
                ]
                UT = epiA.tile([128, 4, S, HT], f16, tag="UT")
                with tc.tile_pool(name="pse", bufs=1, space="PSUM") as pse:
                    with (
                        tc.tile_pool(name="psU", bufs=1, space="PSUM") as psU_pool,
                        tc.tile_pool(name="psD", bufs=1, space="PSUM") as psD_pool,
                        tc.tile_pool(name="psDT", bufs=1, space="PSUM") as psDT_pool,
                        tc.tile_pool(name="fb", bufs=3) as fpool,
                        tc.tile_pool(name="ftb", bufs=3) as ftpool,
                        tc.tile_pool(name="pb", bufs=4) as ppool,
                        tc.tile_pool(name="rb", bufs=2) as rpool,
                        tc.tile_pool(name="psS", bufs=3, space="PSUM") as psS_pool,
                    ):
                        def emit_U(st):
                            PT2p, PT8p, Fcp, qp, sp, pairp, Upsp, dpsp = st
                            for t in range(2):
                                ti = 2 * pairp + t
                                for h2 in range(2):
                                    nc.tensor.matmul(
                                        Upsp[h2][:],
                                        PT2p[:, t, h2 * 128 : (h2 + 1) * 128],
                                        Fcp[:, qp, t, :],
                                        start=(ti == 0),
                                        stop=(ti == NT - 1),
                                    )

                        def finish_scene(st):
                            sp, Upsp, dpsp = st[4], st[6], st[7]
                            # DVE may read only one PSUM operand: stage the
                            # den accumulator in SBUF before folding halves
                            nc.vector.tensor_copy(dacc[sp][:], dpsp[:])
                            nc.vector.tensor_add(
                                den_sb[:, sp, :],
                                dacc[sp][:, 0, :],
                                dacc[sp][:, 1, :],
                            )
                            dT = psDT_pool.tile(
                                [128, 2], f32, tag="dT", name=f"dT{sp}"
                            )
                            for h2 in range(2):
                                nc.tensor.transpose(
                                    dT[:, h2 : h2 + 1],
                                    den_sb[:, sp, h2 * 128 : (h2 + 1) * 128],
                                    id32[:1, :1],
                                )
                            nc.vector.reciprocal(rden[:, :, sp], dT[:])
                            nc.vector.tensor_scalar_mul(
                                Uhat[sp][:, 0, :],
                                Upsp[0][:],
                                rden[:, 0, sp : sp + 1],
                            )
                            nc.scalar.activation(
                                Uhat[sp][:, 1, :],
                                Upsp[1][:],
                                AF.Copy,
                                scale=rden[:, 1, sp : sp + 1],
                            )
                            # transpose Uhat -> UT while the next scene streams
                            ps_u = pse.tile(
                                [128, 4, HT], f16, tag="tre", name=f"ps_u{sp}"
                            )
                            for jc in range(4):
                                for h2 in range(2):
                                    nc.tensor.transpose(
                                        ps_u[:, jc, h2 * 128 : (h2 + 1) * 128],
                                        Uhat[sp][:, h2, jc * 128 : (jc + 1) * 128],
                                        identh[:],
                                    )
                            nc.vector.tensor_copy(
                                UT[:, 0:2, sp, :], ps_u[:, 0:2, :]
                            )
                            nc.vector.tensor_copy(
                                UT[:, 2:4, sp, :], ps_u[:, 2:4, :]
                            )

                        def flush(st):
                            emit_U(st)
                            if st[5] == NP - 1:  # last pair of its scene
                                finish_scene(st)

                        # PE warmup: ~40 independent matmuls on the identity
                        # keep the PE busy through the first chunk DMAs and
                        # bring it to the full-speed p-state before the
                        # real stream begins.
                        warm = psU_pool.tile([128, C], f32, tag="U0", name="warm")
                        for _ in range(28):
                            nc.tensor.matmul(
                                warm[:, 0:128],
                                wsrc[:],
                                wsrc[:],
                                start=True,
                                stop=True,
                            )

                        pend = []
                        for s in range(S):
                            Ups = [
                                psU_pool.tile(
                                    [128, C], f32, tag=f"U{h2}", name=f"U{s}{h2}"
                                )
                                for h2 in range(2)
                            ]
                            dps = psD_pool.tile(
                                [1, 2, HT], f32, tag="den", name=f"d{s}"
                            )
                            for g in range(NCH):
                                FTc = ftpool.tile(
                                    [128, 4, gch * 256], f16, tag="FT"
                                )
                                if s == 0 and g == 0:
                                    # split the very first featT chunk into
                                    # pair pieces so the stream starts early
                                    for q0 in range(gch):
                                        nc.sync.dma_start(
                                            FTc[:, :, q0 * 256 : (q0 + 1) * 256],
                                            featTv[s, g][
                                                :, :, q0 * 256 : (q0 + 1) * 256
                                            ],
                                        )
                                else:
                                    nc.sync.dma_start(FTc[:], featTv[s, g])
                                Fc = fpool.tile([128, gch, 2, C], f16, tag="F")
                                nc.sync.dma_start(Fc[:], featv[s, g])
                                ci = s * NCH + g
                                if ci == 0:
                                    load_epi_small()
                                elif epi_pieces:
                                    epi_pieces.pop(0)()
                                for q in range(gch):
                                    pair = g * gch + q
                                    ps_s = psS_pool.tile(
                                        [128, 2, HT], f32, tag="sc"
                                    )
                                    for t in range(2):
                                        n0 = (2 * q + t) * 128
                                        for j in range(4):
                                            nc.tensor.matmul(
                                                ps_s[:, t, :],
                                                FTc[:, j, n0 : n0 + 128],
                                                akT[:, j, :],
                                                start=(j == 0),
                                                stop=(j == 3),
                                            )
                                    PT2 = ppool.tile([128, 2, HT], f16, tag="PT")
                                    nc.scalar.activation(PT2[:], ps_s[:], AF.Exp)
                                    if DEN_POOL:
                                        red = rpool.tile(
                                            [128, 2, HT], f32, tag="red"
                                        )
                                        nc.gpsimd.partition_all_reduce(
                                            red[:],
                                            PT2[:],
                                            channels=128,
                                            reduce_op=bass_isa.ReduceOp.add,
                                        )
                                        if pair == 0:
                                            nc.vector.tensor_copy(
                                                dacc[s][:], red[0:1, :, :]
                                            )
                                        else:
                                            nc.vector.tensor_add(
                                                dacc[s][:],
                                                dacc[s][:],
                                                red[0:1, :, :],
                                            )
                                    if pend:
                                        st = pend[-1]
                                        nc.tensor.matmul(
                                            st[7][:],
                                            ones_col[:],
                                            st[0][:],
                                            start=(st[5] == 0),
                                            stop=(st[5] == NP - 1),
                                        )
                                    if len(pend) == 2:
                                        flush(pend.pop(0))
                                    pend.append((PT2, PT2, Fc, q, s, pair, Ups, dps))
                        # any weight pieces not yet issued (few chunk
                        # boundaries): issue them now, before the epilogue
                        while epi_pieces:
                            epi_pieces.pop(0)()
                        st = pend[-1]
                        nc.tensor.matmul(
                            st[7][:],
                            ones_col[:],
                            st[0][:],
                            start=(st[5] == 0),
                            stop=(st[5] == NP - 1),
                        )
                        for st in pend:
                            flush(st)
                    # ---- epilogue (streaming PSUM banks now free) ----
                    with (
                        tc.tile_pool(name="epiB", bufs=1) as epi,
                        tc.tile_pool(name="pacc", bufs=1, space="PSUM") as pacc,
                    ):
                        # attention value projection: attnT[(hd), (s,t)].
                        # Two heads share each 128-wide stationary; the
                        # off-diagonal PE quadrants of the output are junk
                        # and simply never read.
                        at_ps = pacc.tile([128, 4, 2, 2 * T], f32, tag="at")
                        for gq in range(4):
                            for jc in range(4):
                                nc.tensor.matmul(
                                    at_ps[:, gq, :, :],
                                    wvT[:, jc, gq * 2 * DH : (gq + 1) * 2 * DH],
                                    UT[:, jc, :, gq * 2 * T : (gq + 1) * 2 * T],
                                    start=(jc == 0),
                                    stop=(jc == 3),
                                )
                        at_sb = epi.tile([128, 4, S, T], f16, tag="at_sb")
                        nc.vector.tensor_copy(
                            at_sb[0:64, :, :, :],
                            at_ps[0:64, :, :, 0:T],
                        )
                        nc.vector.tensor_copy(
                            at_sb[64:128, :, :, :],
                            at_ps[64:128, :, :, T : 2 * T],
                        )

                        # output projection -> h = out + query + b
                        ph = pacc.tile([ST, C], f32, tag="ph")
                        for gq in range(4):
                            nc.tensor.matmul(
                                ph[:],
                                at_sb[:, gq, :, :],
                                woT[:, gq, :],
                                start=(gq == 0),
                                stop=(gq == 3),
                            )
                        h_sb = epi.tile([ST, C], f32, tag="h")
                        nc.vector.tensor_add(h_sb[:], ph[:], qb2[:])

                        # layernorm -> z (fp16): var = E[h^2] - mu^2, then
                        # z = h*rstd - mu*rstd in one fused two-scalar op.
                        ssum = epi.tile([ST, 1], f32, tag="ssum")
                        nc.vector.reduce_sum(
                            ssum[:], h_sb[:], axis=mybir.AxisListType.X
                        )
                        sq = epi.tile([ST, C], f32, tag="sq")
                        s2 = epi.tile([ST, 1], f32, tag="s2")
                        nc.scalar.activation(
                            sq[:], h_sb[:], AF.Square, accum_out=s2[:]
                        )
                        mu = epi.tile([ST, 1], f32, tag="mu")
                        nc.vector.tensor_scalar_mul(mu[:], ssum[:], 1.0 / C)
                        musq = epi.tile([ST, 1], f32, tag="musq")
                        nc.vector.tensor_tensor(musq[:], mu[:], mu[:], op=ALU.mult)
                        var = epi.tile([ST, 1], f32, tag="var")
                        nc.vector.tensor_scalar(
                            var[:],
                            s2[:],
                            1.0 / C,
                            musq[:],
                            op0=ALU.mult,
                            op1=ALU.subtract,
                        )
                        epsc = epi.tile([ST, 1], f32, tag="epsc")
                        nc.vector.memset(epsc[:], 1e-5)
                        std = epi.tile([ST, 1], f32, tag="std")
                        nc.scalar.activation(std[:], var[:], AF.Sqrt, bias=epsc[:])
                        rstd = epi.tile([ST, 1], f32, tag="rstd")
                        nc.vector.reciprocal(rstd[:], std[:])
                        z = epi.tile([ST, C], f16, tag="z")
                        nc.vector.tensor_scalar(
                            z[:],
                            h_sb[:],
                            mu[:],
                            rstd[:],
                            op0=ALU.subtract,
                            op1=ALU.mult,
                        )

                        # bias matmuls first: they depend on nothing and
                        # keep the PE busy while the LayerNorm chain runs
                        pf = [
                            pacc.tile([ST, C], f32, tag=f"pf{half}", name=f"pf{half}")
                            for half in range(2)
                        ]
                        po = pacc.tile([ST, C], f32, tag="po")
                        for half in range(2):
                            nc.tensor.matmul(
                                pf[half][:],
                                ones[:],
                                b1e[:, half * C : (half + 1) * C],
                                start=True,
                                stop=False,
                            )
                        nc.tensor.matmul(po[:], ones[:], b2e[:], start=True, stop=False)

                        # zT
                        zT = epi.tile([128, 4, ST], f16, tag="zT")
                        ps_z = pse.tile([128, 4, ST], f16, tag="tre", name="ps_z")
                        for jc in range(4):
                            nc.tensor.transpose(
                                ps_z[:, jc, :],
                                z[:, jc * 128 : (jc + 1) * 128],
                                identh[:ST, :ST],
                            )
                        nc.vector.tensor_copy(zT[:], ps_z[:])

                        # ff1 + gelu
                        gm = epi.tile([ST, 2, C], f16, tag="gm")
                        for half in range(2):
                            for jc in range(4):
                                nc.tensor.matmul(
                                    pf[half][:],
                                    zT[:, jc, :],
                                    w1gT[:, jc, half * C : (half + 1) * C],
                                    start=False,
                                    stop=(jc == 3),
                                )
                            nc.scalar.activation(gm[:, half, :], pf[half][:], AF.Gelu)

                        # gmT
                        gmT = epi.tile([128, 8, ST], f16, tag="gmT")
                        for half in range(2):
                            ps_g = pse.tile(
                                [128, 4, ST], f16, tag="tre", name=f"ps_g{half}"
                            )
                            for jc in range(4):
                                nc.tensor.transpose(
                                    ps_g[:, jc, :],
                                    gm[:, half, jc * 128 : (jc + 1) * 128],
                                    identh[:ST, :ST],
                                )
                            nc.vector.tensor_copy(
                                gmT[:, half * 4 : (half + 1) * 4, :], ps_g[:]
                            )

                        # ff2 + residual
                        for k in range(8):
                            nc.tensor.matmul(
                                po[:],
                                gmT[:, k, :],
                                w2T[:, k, :],
                                start=False,
                                stop=(k == 7),
                            )
                        fin = epi.tile([ST, C], f32, tag="fin")
                        HF = ST // 2
                        nc.vector.tensor_add(
                            fin[0:HF, :], h_sb[0:HF, :], po[0:HF, :]
                        )
                        nc.sync.dma_start(outp[0:HF, :], fin[0:HF, :])
                        nc.vector.tensor_add(
                            fin[HF:, :], h_sb[HF:, :], po[HF:, :]
                        )
                        nc.sync.dma_start(outp[HF:, :], fin[HF:, :])

    if split:
        _split_multi_waits(nc)
    return nc


def _pick_geometry(max_count):
    """Smallest padded scene length P (and its chunking gch) with
    P = k * 256 * gch >= max_count, preferring larger DMA chunks."""
    best = None
    for gch in (3, 2, 1):
        blk = 256 * gch
        Pg = ((max_count + blk - 1) // blk) * blk
        if best is None or Pg < best[0]:
            best = (Pg, gch)
    return best


def _host_prep(inputs):
    feat = np.asarray(inputs["feat"], dtype=np.float32)
    batch_idx = np.asarray(inputs["batch_idx"]).astype(np.int64)
    B = int(np.asarray(inputs["batch_size"]))
    query = np.asarray(inputs["query"], dtype=np.float32)
    g_q = np.asarray(inputs["g_q"], np.float32)
    b_q = np.asarray(inputs["b_q"], np.float32)
    w_q = np.asarray(inputs["w_q"], np.float32)
    w_k = np.asarray(inputs["w_k"], np.float32)
    w_v = np.asarray(inputs["w_v"], np.float32)
    b_q_in = np.asarray(inputs["b_q_in"], np.float32)
    b_v_in = np.asarray(inputs["b_v_in"], np.float32)
    w_o = np.asarray(inputs["w_o"], np.float32)
    b_o = np.asarray(inputs["b_o"], np.float32)
    g_ff = np.asarray(inputs["g_ff"], np.float32)
    b_ff = np.asarray(inputs["b_ff"], np.float32)
    w1 = np.asarray(inputs["w1"], np.float32)
    b1 = np.asarray(inputs["b1"], np.float32)
    w2 = np.asarray(inputs["w2"], np.float32)
    b2 = np.asarray(inputs["b2"], np.float32)

    S = B // NCORES
    counts = np.bincount(batch_idx, minlength=B)
    offs = np.concatenate([[0], np.cumsum(counts)])
    P, gch = _pick_geometry(int(counts.max()))
    NT = P // 128

    f16 = np.float16

    # query-side fold (host; tiny)
    q = query[0]
    mu = q.mean(-1, keepdims=True)
    var = ((q - mu) ** 2).mean(-1, keepdims=True)
    qn = (q - mu) / np.sqrt(var + 1e-5) * g_q + b_q
    qh = (qn @ w_q.T + b_q_in) / np.sqrt(DH)  # [T, C]
    A = np.einsum(
        "thd,hdc->cht", qh.reshape(T, H, DH), w_k.reshape(H, DH, C)
    ).reshape(C, H * T)

    # pad-column vector v with v @ A[:, ht] <= -22 for every ht: padded
    # featT columns score ~exp(-22)=0, so the denominator needs no mask.
    An = A / np.linalg.norm(A, axis=0, keepdims=True)
    u, *_ = np.linalg.lstsq(An.T, np.ones(H * T), rcond=None)
    if (An.T @ u).min() < 0.5:
        raise RuntimeError("pad-vector separation failed")

    featp = np.zeros((NCORES, S * P, C), dtype=f16)
    for b in range(B):
        c, s = divmod(b, S)
        n = counts[b]
        featp[c, s * P : s * P + n] = feat[offs[b] : offs[b + 1]].astype(f16)
    featTp = np.ascontiguousarray(featp.transpose(0, 2, 1))  # [NCORES, C, S*P]
    m = float((A.T @ u).min())
    v = (-(22.0 / m) * u).astype(f16)
    for b in range(B):
        c, s = divmod(b, S)
        n = counts[b]
        if n < P:
            featTp[c, :, s * P + n : (s + 1) * P] = v[:, None]

    consts = dict(
        akT=np.ascontiguousarray(A.astype(f16)),
        wvT=np.ascontiguousarray(w_v.T.astype(f16)),
        woT=np.ascontiguousarray(w_o.T.astype(f16)),
        w1gT=np.ascontiguousarray((w1 * g_ff[None, :]).T.astype(f16)),
        b1e=(b1 + w1 @ b_ff).reshape(1, 2 * C).astype(f16),
        w2T=np.ascontiguousarray(w2.T.astype(f16)),
        b2e=b2.reshape(1, C).astype(f16),
        qb=np.ascontiguousarray(query[0] + (b_o + w_o @ b_v_in)[None, :]).astype(
            np.float32
        ),
        identh=np.eye(128, dtype=f16),
        ident32=np.eye(128, dtype=np.float32),
    )
    in_maps = []
    for c in range(NCORES):
        m = dict(consts)
        m["featp"] = featp[c]
        m["featTp"] = featTp[c]
        in_maps.append(m)
    return in_maps, P, S, B, gch


def kernel(**inputs):
    from concourse.bass_utils import run_bass_kernel_spmd

    in_maps, P, S, B, gch = _host_prep(inputs)
    key = (P, S, gch)
    if key not in _CACHE:
        _CACHE[key] = _build(P, S, gch=gch)
    nc = _CACHE[key]
    res = run_bass_kernel_spmd(nc, in_maps, core_ids=list(range(NCORES)))
    out = np.empty((B, T, C), dtype=np.float32)
    for c in range(NCORES):
        o = res.results[c]["outp"]
        for s in range(S):
            out[c * S + s] = o[s * T : (s + 1) * T]
    return out


# revision 51
# speedup vs baseline: 1.1860x; 1.1701x over previous
"""Trainium2 Bass kernel for nn_ConditionPooler (ragged cross-attention pooler).

Algorithm (per core, data-parallel over B=16 scenes, S=2 scenes/core on 8 cores):
  scores^T[n,(h,t)] = feat @ A         where A[c,(h,t)] = sum_d qh[t,h,d] w_k[h*DH+d,c]
  P = exp(scores)  (no max-subtraction; scores in [-9, 9], fp16 exp is exact
                    enough; b_k_in cancels in softmax)
  U[(h,t),c]  = sum_n P[n,(h,t)] feat[n,c]   (per scene; padded rows are 0)
  den[(h,t)]  = sum_n P[n,(h,t)] mask[n]
  Uhat = U / den; attn_h = Uhat_h @ w_v_h^T; out = attn @ w_o^T + b_o (+ w_o b_v_in)
  h = out + query; z = (h-mu)/std; ff = gelu(z @ (w1*g)^T + b1eff) @ w2^T + b2
  result = h + ff

Performance design (v2): the previous build was PE-sequencer bound (~124ns of
PE.SEQ per matmul call, 515 calls).  This build:
  - feat is DMA'd in BOTH orientations (row-major fp16 for the U matmul's
    moving operand, transposed fp16 for the scores matmul's stationary
    operand) so there are NO on-chip PE transposes in the stream.
  - fp16 streaming operands (quantization error ~6e-4, final rel err ~baseline).
  - 7 matmul calls per 128-row tile: 4 scores (c-block chains), 2 U (ht
    halves), 1 denominator (mask-stationary, accumulating [1, 256] per scene).
  - U/den calls for pair p are emitted after the scores of pair p+1 so the
    PE never stalls on the exp (Activation engine) latency.
Scenes are padded to a common length P (multiple of 256*gch) so the SPMD
program is static; segment boundaries come from batch_idx on the host.
"""

import numpy as np

DEN_POOL = False

C = 512
T = 32
H = 8
DH = C // H
NCORES = 8

_CACHE = {}


def _apply_tile_patch():
    """This walrus build allows only one sem wait on CTRL-encoded (Drain)
    instructions; TileContext's tail drain carries the whole global clock.
    Split the extra waits onto standalone sync-engine nops."""
    import concourse.tile as tile_mod
    import concourse.mybir as mybir
    from concourse.vector_clock import ScopedClock

    if getattr(tile_mod.TileContext, "_drain_patched", False):
        return

    def _patched(self, tick_clock, wait_clock):
        nc = self.nc
        drain_inst = nc.sync.drain()
        wait_clock.add_sem_waits(
            drain_inst.ins, ScopedClock({None: tick_clock.global_clock})
        )
        si = drain_inst.ins.sync_info
        if si is not None and si.on_wait is not None and len(si.on_wait) > 1:
            waits = list(si.on_wait)
            si.on_wait = waits[:1]
            for w in waits[1:]:
                nop = nc.sync.nop(nofuse=True)
                nsi = nop.ins.sync_info
                if nsi is None:
                    nop.ins.sync_info = mybir.SyncInfo(on_wait=[w], on_update=[])
                else:
                    nsi.on_wait = [w]
        nc.all_engine_barrier()
        assert self.sems is not None
        popped = nc._tile_sem_poison_stack.pop()
        assert popped is self._sem_poison
        nc.clear_and_free_semaphores(list(self.sems.allocated().values()))
        nc.all_engine_barrier()

    tile_mod.TileContext._drain_and_barrier = _patched
    tile_mod.TileContext._drain_patched = True


def _split_multi_waits(nc):
    """This walrus build caps sync waits at 1 per instruction (2 for
    EventSemaphore). Tile emits several on some instructions; hoist the
    extras onto same-engine NoOps inserted just before."""
    import concourse.mybir as mybir

    cnt = [0]
    for f in nc.m.functions:
        for b in f.blocks:
            newlist = []
            for inst in b.instructions:
                si = inst.sync_info
                if si is not None and si.on_wait is not None and len(si.on_wait) > 1:
                    waits = list(si.on_wait)
                    for w in waits[:-1]:
                        cnt[0] += 1
                        nop = mybir.InstNoOp(
                            name=f"I-wsplit-{cnt[0]}", ins=[], outs=[]
                        )
                        nop.engine = inst.engine
                        nop.sync_info = mybir.SyncInfo(on_wait=[w], on_update=[])
                        newlist.append(nop)
                    si.on_wait = waits[-1:]
                newlist.append(inst)
            b.instructions = newlist


def _build(P, S, gch=3, split=True):
    """Build the per-core SPMD Bass program. P = padded scene length
    (multiple of 256*gch), S = scenes per core, gch = tile-pairs per DMA
    chunk."""
    import concourse.bass as bass
    import concourse.bass_isa as bass_isa
    import concourse.mybir as mybir
    import concourse.tile as tile

    _apply_tile_patch()

    f32 = mybir.dt.float32
    f16 = mybir.dt.float16
    HT = H * T  # 256
    NT = P // 128
    NP = NT // 2  # tile pairs per scene
    NCH = NP // gch  # DMA chunks per scene
    assert P % (256 * gch) == 0
    AF = mybir.ActivationFunctionType
    ALU = mybir.AluOpType

    nc = bass.Bass()
    f8 = mybir.dt.float8e3
    f8e4 = mybir.dt.float8e4
    featp = nc.dram_tensor("featp", [S * P, C], f16, kind="ExternalInput")
    featTp = nc.dram_tensor("featTp", [C, S * P], f16, kind="ExternalInput")
    akT_d = nc.dram_tensor("akT", [C, HT], f16, kind="ExternalInput")
    wvT_d = nc.dram_tensor("wvT", [C, C], f16, kind="ExternalInput")
    woT_d = nc.dram_tensor("woT", [C, C], f16, kind="ExternalInput")
    w1gT_d = nc.dram_tensor("w1gT", [C, 2 * C], f16, kind="ExternalInput")
    b1e_d = nc.dram_tensor("b1e", [1, 2 * C], f16, kind="ExternalInput")
    w2T_d = nc.dram_tensor("w2T", [2 * C, C], f16, kind="ExternalInput")
    b2e_d = nc.dram_tensor("b2e", [1, C], f16, kind="ExternalInput")
    qb_d = nc.dram_tensor("qb", [T, C], f32, kind="ExternalInput")
    idh_d = nc.dram_tensor("identh", [128, 128], f16, kind="ExternalInput")
    id32_d = nc.dram_tensor("ident32", [128, 128], f32, kind="ExternalInput")
    outp = nc.dram_tensor("outp", [S * T, C], f32, kind="ExternalOutput")

    ST = S * T

    with tile.TileContext(nc) as tc:
        with tc.tile_pool(name="const", bufs=1) as const:
            # akT first on the sync queue (scores need it); the warmup
            # matmuls run on a memset tile so they need no DMA at all
            akT = const.tile([128, 4, HT], f16, tag="akT")
            nc.sync.dma_start(akT[:], akT_d.rearrange("(j p) f -> p j f", p=128))
            identh = const.tile([128, 128], f16, tag="identh")
            wsrc = const.tile([128, 128], f16, tag="wsrc")
            nc.vector.memset(wsrc[:], 0.001)
            wvT = const.tile([128, 4, C], f16, tag="wvT")
            woT = const.tile([128, 4, C], f16, tag="woT")
            w1gT = const.tile([128, 4, 2 * C], f16, tag="w1gT")
            w2T = const.tile([128, 8, C], f16, tag="w2T")
            b1e = const.tile([1, 2 * C], f16, tag="b1e")
            b2e = const.tile([1, C], f16, tag="b2e")
            qb2 = const.tile([ST, C], f32, tag="qb2")
            id32 = const.tile([128, 128], f32, tag="id32")

            # Epilogue weights are loaded in pieces, one per feat-chunk
            # boundary on the same (in-order) queue as the feat stream, so
            # they never displace more than one chunk-slack of feat DMA.
            wv_v = wvT_d.rearrange("(j p) f -> p j f", p=128)
            wo_v = woT_d.rearrange("(j p) f -> p j f", p=128)
            w1_v = w1gT_d.rearrange("(j p) f -> p j f", p=128)
            w2_v = w2T_d.rearrange("(j p) f -> p j f", p=128)
            epi_pieces = [
                lambda: nc.sync.dma_start(wvT[:], wv_v),
                lambda: nc.sync.dma_start(woT[:], wo_v),
                lambda: nc.sync.dma_start(w1gT[:, :, 0:C], w1_v[:, :, 0:C]),
                lambda: nc.sync.dma_start(w1gT[:, :, C:], w1_v[:, :, C:]),
                lambda: nc.sync.dma_start(w2T[:, 0:4, :], w2_v[:, 0:4, :]),
                lambda: nc.sync.dma_start(w2T[:, 4:8, :], w2_v[:, 4:8, :]),
            ]

            def load_epi_small():
                g = nc.gpsimd
                g.dma_start(b1e[:], b1e_d[:])
                g.dma_start(b2e[:], b2e_d[:])
                g.dma_start(id32[:], id32_d[:])
                g.dma_start(identh[:], idh_d[:])
                for s in range(S):
                    g.dma_start(qb2[s * T : (s + 1) * T, :], qb_d[:])

            ones_col = const.tile([128, 1], f16, tag="ones_col")
            nc.vector.memset(ones_col[:], 1.0)
            ones = const.tile([1, ST], f16, tag="ones")
            nc.vector.memset(ones[:], 1.0)

            featv = featp.rearrange(
                "(s g q t p) c -> s g p q t c", p=128, t=2, q=gch, g=NCH
            )
            featTv = featTp.rearrange(
                "(j p) (s g n) -> s g p j n", p=128, g=NCH, n=gch * 256
            )

            with tc.tile_pool(name="epiA", bufs=1) as epiA:
                rden = epiA.tile([128, 2, S], f32, tag="rden")
                den_sb = epiA.tile([1, S, HT], f32, tag="den_sb")
                dacc = [
                    epiA.tile([1, 2, HT], f32, tag=f"dacc{s}", name=f"dacc{s}")
                    for s in range(S)
                ]
                Uhat = [
                    epiA.tile([128, 2, C], f16, tag=f"Uh{s}", name=f"Uh{s}")
                    for s in range(S)
                ]
                UT = epiA.tile([128, 4, S, HT], f16, tag="UT")
                with tc.tile_pool(name="pse", bufs=1, space="PSUM") as pse:
                    with (
                        tc.tile_pool(name="psU", bufs=1, space="PSUM") as psU_pool,
                        tc.tile_pool(name="psD", bufs=1, space="PSUM") as psD_pool,
                        tc.tile_pool(name="psDT", bufs=1, space="PSUM") as psDT_pool,
                        tc.tile_pool(name="fb", bufs=3) as fpool,
                        tc.tile_pool(name="ftb", bufs=3) as ftpool,
                        tc.tile_pool(name="pb", bufs=4) as ppool,
                        tc.tile_pool(name="rb", bufs=2) as rpool,
                        tc.tile_pool(name="psS", bufs=3, space="PSUM") as psS_pool,
                    ):
                        def emit_U(st):
                            PT2p, PT8p, Fcp, qp, sp, pairp, Upsp, dpsp = st
                            for t in range(2):
                                ti = 2 * pairp + t
                                for h2 in range(2):
                                    nc.tensor.matmul(
                                        Upsp[h2][:],
                                        PT2p[:, t, h2 * 128 : (h2 + 1) * 128],
                                        Fcp[:, qp, t, :],
                                        start=(ti == 0),
                                        stop=(ti == NT - 1),
                                    )

                        def finish_scene(st):
                            sp, Upsp, dpsp = st[4], st[6], st[7]
                            # DVE may read only one PSUM operand: stage the
                            # den accumulator in SBUF before folding halves
                            nc.vector.tensor_copy(dacc[sp][:], dpsp[:])
                            nc.vector.tensor_add(
                                den_sb[:, sp, :],
                                dacc[sp][:, 0, :],
                                dacc[sp][:, 1, :],
                            )
                            dT = psDT_pool.tile(
                                [128, 2], f32, tag="dT", name=f"dT{sp}"
                            )
                            for h2 in range(2):
                                nc.tensor.transpose(
                                    dT[:, h2 : h2 + 1],
                                    den_sb[:, sp, h2 * 128 : (h2 + 1) * 128],
                                    id32[:1, :1],
                                )
                            nc.vector.reciprocal(rden[:, :, sp], dT[:])
                            nc.vector.tensor_scalar_mul(
                                Uhat[sp][:, 0, :],
                                Upsp[0][:],
                                rden[:, 0, sp : sp + 1],
                            )
                            nc.scalar.activation(
                                Uhat[sp][:, 1, :],
                                Upsp[1][:],
                                AF.Copy,
                                scale=rden[:, 1, sp : sp + 1],
                            )
                            # transpose Uhat -> UT while the next scene streams
                            ps_u = pse.tile(
                                [128, 4, HT], f16, tag="tre", name=f"ps_u{sp}"
                            )
                            for jc in range(4):
                                for h2 in range(2):
                                    nc.tensor.transpose(
                                        ps_u[:, jc, h2 * 128 : (h2 + 1) * 128],
                                        Uhat[sp][:, h2, jc * 128 : (jc + 1) * 128],
                                        identh[:],
                                    )
                            nc.vector.tensor_copy(
                                UT[:, 0:2, sp, :], ps_u[:, 0:2, :]
                            )
                            nc.vector.tensor_copy(
                                UT[:, 2:4, sp, :], ps_u[:, 2:4, :]
                            )

                        def flush(st):
                            emit_U(st)
                            if st[5] == NP - 1:  # last pair of its scene
                                finish_scene(st)

                        # PE warmup: ~40 independent matmuls on the identity
                        # keep the PE busy through the first chunk DMAs and
                        # bring it to the full-speed p-state before the
                        # real stream begins.
                        warm = psU_pool.tile([128, C], f32, tag="U0", name="warm")
                        for _ in range(28):
                            nc.tensor.matmul(
                                warm[:, 0:128],
                                wsrc[:],
                                wsrc[:],
                                start=True,
                                stop=True,
                            )

                        pend = []
                        for s in range(S):
                            Ups = [
                                psU_pool.tile(
                                    [128, C], f32, tag=f"U{h2}", name=f"U{s}{h2}"
                                )
                                for h2 in range(2)
                            ]
                            dps = psD_pool.tile(
                                [1, 2, HT], f32, tag="den", name=f"d{s}"
                            )
                            for g in range(NCH):
                                FTc = ftpool.tile(
                                    [128, 4, gch * 256], f16, tag="FT"
                                )
                                if s == 0 and g == 0:
                                    # split the very first featT chunk into
                                    # pair pieces so the stream starts early
                                    for q0 in range(gch):
                                        nc.sync.dma_start(
                                            FTc[:, :, q0 * 256 : (q0 + 1) * 256],
                                            featTv[s, g][
                                                :, :, q0 * 256 : (q0 + 1) * 256
                                            ],
                                        )
                                else:
                                    nc.sync.dma_start(FTc[:], featTv[s, g])
                                Fc = fpool.tile([128, gch, 2, C], f16, tag="F")
                                nc.sync.dma_start(Fc[:], featv[s, g])
                                ci = s * NCH + g
                                if ci == 0:
                                    load_epi_small()
                                elif epi_pieces:
                                    epi_pieces.pop(0)()
                                for q in range(gch):
                                    pair = g * gch + q
                                    ps_s = psS_pool.tile(
                                        [128, 2, HT], f32, tag="sc"
                                    )
                                    for t in range(2):
                                        n0 = (2 * q + t) * 128
                                        for j in range(4):
                                            nc.tensor.matmul(
                                                ps_s[:, t, :],
                                                FTc[:, j, n0 : n0 + 128],
                                                akT[:, j, :],
                                                start=(j == 0),
                                                stop=(j == 3),
                                            )
                                    PT2 = ppool.tile([128, 2, HT], f16, tag="PT")
                                    nc.scalar.activation(PT2[:], ps_s[:], AF.Exp)
                                    if DEN_POOL:
                                        red = rpool.tile(
                                            [128, 2, HT], f32, tag="red"
                                        )
                                        nc.gpsimd.partition_all_reduce(
                                            red[:],
                                            PT2[:],
                                            channels=128,
                                            reduce_op=bass_isa.ReduceOp.add,
                                        )
                                        if pair == 0:
                                            nc.vector.tensor_copy(
                                                dacc[s][:], red[0:1, :, :]
                                            )
                                        else:
                                            nc.vector.tensor_add(
                                                dacc[s][:],
                                                dacc[s][:],
                                                red[0:1, :, :],
                                            )
                                    if pend:
                                        st = pend[-1]
                                        nc.tensor.matmul(
                                            st[7][:],
                                            ones_col[:],
                                            st[0][:],
                                            start=(st[5] == 0),
                                            stop=(st[5] == NP - 1),
                                        )
                                    if len(pend) == 2:
                                        flush(pend.pop(0))
                                    pend.append((PT2, PT2, Fc, q, s, pair, Ups, dps))
                        # any weight pieces not yet issued (few chunk
                        # boundaries): issue them now, before the epilogue
                        while epi_pieces:
                            epi_pieces.pop(0)()
                        st = pend[-1]
                        nc.tensor.matmul(
                            st[7][:],
                            ones_col[:],
                            st[0][:],
                            start=(st[5] == 0),
                            stop=(st[5] == NP - 1),
                        )
                        for st in pend:
                            flush(st)
                    # ---- epilogue (streaming PSUM banks now free) ----
                    with (
                        tc.tile_pool(name="epiB", bufs=1) as epi,
                        tc.tile_pool(name="pacc", bufs=1, space="PSUM") as pacc,
                    ):
                        # attention value projection: attnT[(hd), (s,t)].
                        # Two heads share each 128-wide stationary; the
                        # off-diagonal PE quadrants of the output are junk
                        # and simply never read.
                        # two physical half-tiles so the first half's
                        # copy + wo chain overlaps the second half's wv
                        at_ps = [
                            pacc.tile(
                                [128, 2, 2, 2 * T], f32, tag=f"at{h}", name=f"at{h}"
                            )
                            for h in range(2)
                        ]
                        at_sb = [
                            epi.tile(
                                [128, 2, S, T], f16, tag=f"atsb{h}", name=f"atsb{h}"
                            )
                            for h in range(2)
                        ]
                        for gq in range(4):
                            half, g2 = divmod(gq, 2)
                            for jc in range(4):
                                nc.tensor.matmul(
                                    at_ps[half][:, g2, :, :],
                                    wvT[:, jc, gq * 2 * DH : (gq + 1) * 2 * DH],
                                    UT[:, jc, :, gq * 2 * T : (gq + 1) * 2 * T],
                                    start=(jc == 0),
                                    stop=(jc == 3),
                                )
                            if g2 == 1:
                                nc.vector.tensor_copy(
                                    at_sb[half][0:64, :, :, :],
                                    at_ps[half][0:64, :, :, 0:T],
                                )
                                nc.vector.tensor_copy(
                                    at_sb[half][64:128, :, :, :],
                                    at_ps[half][64:128, :, :, T : 2 * T],
                                )

                        # output projection -> h = out + query + b
                        ph = pacc.tile([ST, C], f32, tag="ph")
                        for gq in range(4):
                            half, g2 = divmod(gq, 2)
                            nc.tensor.matmul(
                                ph[:],
                                at_sb[half][:, g2, :, :],
                                woT[:, gq, :],
                                start=(gq == 0),
                                stop=(gq == 3),
                            )
                        h_sb = epi.tile([ST, C], f32, tag="h")
                        nc.vector.tensor_add(h_sb[:], ph[:], qb2[:])

                        # layernorm -> z (fp16): var = E[h^2] - mu^2, then
                        # z = h*rstd - mu*rstd in one fused two-scalar op.
                        ssum = epi.tile([ST, 1], f32, tag="ssum")
                        nc.vector.reduce_sum(
                            ssum[:], h_sb[:], axis=mybir.AxisListType.X
                        )
                        sq = epi.tile([ST, C], f32, tag="sq")
                        s2 = epi.tile([ST, 1], f32, tag="s2")
                        nc.scalar.activation(
                            sq[:], h_sb[:], AF.Square, accum_out=s2[:]
                        )
                        mu = epi.tile([ST, 1], f32, tag="mu")
                        nc.vector.tensor_scalar_mul(mu[:], ssum[:], 1.0 / C)
                        musq = epi.tile([ST, 1], f32, tag="musq")
                        nc.vector.tensor_tensor(musq[:], mu[:], mu[:], op=ALU.mult)
                        var = epi.tile([ST, 1], f32, tag="var")
                        nc.vector.tensor_scalar(
                            var[:],
                            s2[:],
                            1.0 / C,
                            musq[:],
                            op0=ALU.mult,
                            op1=ALU.subtract,
                        )
                        epsc = epi.tile([ST, 1], f32, tag="epsc")
                        nc.vector.memset(epsc[:], 1e-5)
                        std = epi.tile([ST, 1], f32, tag="std")
                        nc.scalar.activation(std[:], var[:], AF.Sqrt, bias=epsc[:])
                        rstd = epi.tile([ST, 1], f32, tag="rstd")
                        nc.vector.reciprocal(rstd[:], std[:])
                        z = epi.tile([ST, C], f16, tag="z")
                        nc.vector.tensor_scalar(
                            z[:],
                            h_sb[:],
                            mu[:],
                            rstd[:],
                            op0=ALU.subtract,
                            op1=ALU.mult,
                        )

                        # bias matmuls first: they depend on nothing and
                        # keep the PE busy while the LayerNorm chain runs
                        pf = [
                            pacc.tile([ST, C], f32, tag=f"pf{half}", name=f"pf{half}")
                            for half in range(2)
                        ]
                        po = pacc.tile([ST, C], f32, tag="po")
                        for half in range(2):
                            nc.tensor.matmul(
                                pf[half][:],
                                ones[:],
                                b1e[:, half * C : (half + 1) * C],
                                start=True,
                                stop=False,
                            )
                        nc.tensor.matmul(po[:], ones[:], b2e[:], start=True, stop=False)

                        # zT
                        zT = epi.tile([128, 4, ST], f16, tag="zT")
                        ps_z = pse.tile([128, 4, ST], f16, tag="tre", name="ps_z")
                        for jc in range(4):
                            nc.tensor.transpose(
                                ps_z[:, jc, :],
                                z[:, jc * 128 : (jc + 1) * 128],
                                identh[:ST, :ST],
                            )
                        nc.vector.tensor_copy(zT[:], ps_z[:])

                        # ff1 + gelu
                        gm = epi.tile([ST, 2, C], f16, tag="gm")
                        for half in range(2):
                            for jc in range(4):
                                nc.tensor.matmul(
                                    pf[half][:],
                                    zT[:, jc, :],
                                    w1gT[:, jc, half * C : (half + 1) * C],
                                    start=False,
                                    stop=(jc == 3),
                                )
                            nc.scalar.activation(gm[:, half, :], pf[half][:], AF.Gelu)

                        # gmT
                        gmT = epi.tile([128, 8, ST], f16, tag="gmT")
                        for half in range(2):
                            ps_g = pse.tile(
                                [128, 4, ST], f16, tag="tre", name=f"ps_g{half}"
                            )
                            for jc in range(4):
                                nc.tensor.transpose(
                                    ps_g[:, jc, :],
                                    gm[:, half, jc * 128 : (jc + 1) * 128],
                                    identh[:ST, :ST],
                                )
                            nc.vector.tensor_copy(
                                gmT[:, half * 4 : (half + 1) * 4, :], ps_g[:]
                            )

                        # ff2 + residual
                        for k in range(8):
                            nc.tensor.matmul(
                                po[:],
                                gmT[:, k, :],
                                w2T[:, k, :],
                                start=False,
                                stop=(k == 7),
                            )
                        fin = epi.tile([ST, C], f32, tag="fin")
                        HF = ST // 2
                        nc.vector.tensor_add(
                            fin[0:HF, :], h_sb[0:HF, :], po[0:HF, :]
                        )
                        nc.sync.dma_start(outp[0:HF, :], fin[0:HF, :])
                        nc.vector.tensor_add(
                            fin[HF:, :], h_sb[HF:, :], po[HF:, :]
                        )
                        nc.sync.dma_start(outp[HF:, :], fin[HF:, :])

    if split:
        _split_multi_waits(nc)
    return nc


def _pick_geometry(max_count):
    """Smallest padded scene length P (and its chunking gch) with
    P = k * 256 * gch >= max_count, preferring larger DMA chunks."""
    best = None
    for gch in (3, 2, 1):
        blk = 256 * gch
        Pg = ((max_count + blk - 1) // blk) * blk
        if best is None or Pg < best[0]:
            best = (Pg, gch)
    return best


def _host_prep(inputs):
    feat = np.asarray(inputs["feat"], dtype=np.float32)
    batch_idx = np.asarray(inputs["batch_idx"]).astype(np.int64)
    B = int(np.asarray(inputs["batch_size"]))
    query = np.asarray(inputs["query"], dtype=np.float32)
    g_q = np.asarray(inputs["g_q"], np.float32)
    b_q = np.asarray(inputs["b_q"], np.float32)
    w_q = np.asarray(inputs["w_q"], np.float32)
    w_k = np.asarray(inputs["w_k"], np.float32)
    w_v = np.asarray(inputs["w_v"], np.float32)
    b_q_in = np.asarray(inputs["b_q_in"], np.float32)
    b_v_in = np.asarray(inputs["b_v_in"], np.float32)
    w_o = np.asarray(inputs["w_o"], np.float32)
    b_o = np.asarray(inputs["b_o"], np.float32)
    g_ff = np.asarray(inputs["g_ff"], np.float32)
    b_ff = np.asarray(inputs["b_ff"], np.float32)
    w1 = np.asarray(inputs["w1"], np.float32)
    b1 = np.asarray(inputs["b1"], np.float32)
    w2 = np.asarray(inputs["w2"], np.float32)
    b2 = np.asarray(inputs["b2"], np.float32)

    S = B // NCORES
    counts = np.bincount(batch_idx, minlength=B)
    offs = np.concatenate([[0], np.cumsum(counts)])
    P, gch = _pick_geometry(int(counts.max()))
    NT = P // 128

    f16 = np.float16

    # query-side fold (host; tiny)
    q = query[0]
    mu = q.mean(-1, keepdims=True)
    var = ((q - mu) ** 2).mean(-1, keepdims=True)
    qn = (q - mu) / np.sqrt(var + 1e-5) * g_q + b_q
    qh = (qn @ w_q.T + b_q_in) / np.sqrt(DH)  # [T, C]
    A = np.einsum(
        "thd,hdc->cht", qh.reshape(T, H, DH), w_k.reshape(H, DH, C)
    ).reshape(C, H * T)

    # pad-column vector v with v @ A[:, ht] <= -22 for every ht: padded
    # featT columns score ~exp(-22)=0, so the denominator needs no mask.
    An = A / np.linalg.norm(A, axis=0, keepdims=True)
    u, *_ = np.linalg.lstsq(An.T, np.ones(H * T), rcond=None)
    if (An.T @ u).min() < 0.5:
        raise RuntimeError("pad-vector separation failed")

    featp = np.zeros((NCORES, S * P, C), dtype=f16)
    for b in range(B):
        c, s = divmod(b, S)
        n = counts[b]
        featp[c, s * P : s * P + n] = feat[offs[b] : offs[b + 1]].astype(f16)
    featTp = np.ascontiguousarray(featp.transpose(0, 2, 1))  # [NCORES, C, S*P]
    m = float((A.T @ u).min())
    v = (-(22.0 / m) * u).astype(f16)
    for b in range(B):
        c, s = divmod(b, S)
        n = counts[b]
        if n < P:
            featTp[c, :, s * P + n : (s + 1) * P] = v[:, None]

    consts = dict(
        akT=np.ascontiguousarray(A.astype(f16)),
        wvT=np.ascontiguousarray(w_v.T.astype(f16)),
        woT=np.ascontiguousarray(w_o.T.astype(f16)),
        w1gT=np.ascontiguousarray((w1 * g_ff[None, :]).T.astype(f16)),
        b1e=(b1 + w1 @ b_ff).reshape(1, 2 * C).astype(f16),
        w2T=np.ascontiguousarray(w2.T.astype(f16)),
        b2e=b2.reshape(1, C).astype(f16),
        qb=np.ascontiguousarray(query[0] + (b_o + w_o @ b_v_in)[None, :]).astype(
            np.float32
        ),
        identh=np.eye(128, dtype=f16),
        ident32=np.eye(128, dtype=np.float32),
    )
    in_maps = []
    for c in range(NCORES):
        m = dict(consts)
        m["featp"] = featp[c]
        m["featTp"] = featTp[c]
        in_maps.append(m)
    return in_maps, P, S, B, gch


def kernel(**inputs):
    from concourse.bass_utils import run_bass_kernel_spmd

    in_maps, P, S, B, gch = _host_prep(inputs)
    key = (P, S, gch)
    if key not in _CACHE:
        _CACHE[key] = _build(P, S, gch=gch)
    nc = _CACHE[key]
    res = run_bass_kernel_spmd(nc, in_maps, core_ids=list(range(NCORES)))
    out = np.empty((B, T, C), dtype=np.float32)
    for c in range(NCORES):
        o = res.results[c]["outp"]
        for s in range(S):
            out[c * S + s] = o[s * T : (s + 1) * T]
    return out
